# revision 18
# baseline (speedup 1.0000x reference)
"""Trainium2 Bass kernel for the BAN (bilinear attention network) problem.

Math (per batch b, eval mode):
    hq = emb[he_ques] @ Wq + bq                  [NQ, H]
    hk = emb[he_kg]   @ Wk + bk                  [NK, H]
    logits[g,q,k] = sum_d hq[q,d] Watt[d,g] hk[k,d]   (+ batt[g], cancels in
                                                       the joint softmax)
    att = softmax over flattened (q,k) per (b,g)
    pooled[g,d] = sum_{q,k} hq[q,d] att[g,q,k] hk[k,d]
    out = pooled.flat @ Wout + bout;  sim = out @ glove.T;  log_softmax(sim)

Distribution: pure data parallel over batch, 8 samples per core on 8 cores.
All weights replicated. No collectives.

v2 design notes (vs the fp32r baseline):
  - All matmul operands are bf16.  fp32/fp32r stationary operands force a
    slow serial weight load into the PE for every matmul (measured ~200ns
    each, 157us total); bf16 enables FWL and pull-ahead so matmuls run at
    ~N-cycle streaming cost.  Accumulation stays fp32 in PSUM.  Measured
    headroom: tolerance is 2e-2, fp32r baseline error was 8.8e-5.
  - All transposes (X rows -> X.T, hk -> hk.T, out -> out.T) are PE
    transposes in bf16 (1 cycle/row + FWL weight loads), batched 4-to-a-PSUM
    tile so PSUM->SBUF copies are [128, 512] DVE 2x copies.  (The DMA XBAR
    transpose measured ~0.4us per 16x128 tile on HW -- 25x the cost-model
    estimate -- and serialized the whole kernel; the 2-column indirect
    gather also returned wrong data on HW.  Both were reverted.)
  - The final sim matrix is computed *transposed* ([a-tile partitions, batch]
    via glove-stationary matmuls) so log-softmax runs on 128 partitions
    instead of 8 (the [8, 4000] layout was partition-starved).
  - E (=300, +1 bias row) is zero-padded to 384 so every contraction chunk is
    a full 128 rows; N_ANS is padded 4000->4096 (pad exp terms masked to 0).

Layouts (per core, BL=8 samples, partition dim first):
  xT   [128, 18, 3, 128]  X.T     (e-in-chunk, tok-tile, e-chunk, tok)
  hqT  [128, 8, 256]      hq.T    (d-in-tile, m, q-tok)
  hk   [128, 16, 1024]    hk      (k-tok, k-tile, d)
  hkT  [128, 16, 8, 128]  hk.T    (d-in-tile, k-tile, m, k-tok)
  hqw  [128, 8, 8, 32]    hq.T*Watt broadcast over g     (per sample)
  et   [128, 2, 256]      exp(logits.T)  (k-tok, kt, (g,q))
  poT  [128, 8, 8, 8]     pooled.T (d-in-tile, m, g, b)
  simT [128, 32, 8] PSUM  sim.T   (ans-in-tile, ans-tile, b)
"""

import sys

if "/opt/trn_rl_repo" not in sys.path:
    sys.path.insert(0, "/opt/trn_rl_repo")

import numpy as np
import ml_dtypes

import concourse.bass as bass
import concourse.mybir as mybir
import concourse.tile as tile
from concourse import bacc
from concourse.bass_utils import run_bass_kernel_spmd

F32 = mybir.dt.float32
BF16 = mybir.dt.bfloat16
I32 = mybir.dt.int32
AX = mybir.AxisListType
OP = mybir.AluOpType
AF = mybir.ActivationFunctionType

N_CORES = 8
VOCAB = 20000
E = 300          # word embedding size
EAP = 384        # padded (bias row at 300, zeros 301..383); 3 chunks of 128
H = 1024         # hidden
G = 8            # heads
N_OUT = 300
N_ANS = 4000
NA_PAD = 4096    # padded answers; 32 tiles of 128
NA_T = 32
B, NQ, NK = 64, 32, 256
BL = B // N_CORES            # 8 samples per core
TQ = BL * NQ                 # 256 q tokens per core (2 tiles)
TK = BL * NK                 # 2048 k tokens per core (16 tiles)
TQ_TILES = TQ // 128         # 2
TK_TILES = TK // 128         # 16
NT = TQ_TILES + TK_TILES     # 18 gathered token tiles
DT = H // 128                # 8 d-tiles
SIM_SHIFT = -40.0            # exp(sim + SIM_SHIFT): overflow guard


def build_kernel():
    nc = bacc.Bacc("TRN2", target_bir_lowering=False, debug=False,
                   num_devices=N_CORES)

    # ---- DRAM I/O ----
    emb_d = nc.dram_tensor("emb", [VOCAB, EAP], BF16, kind="ExternalInput").ap()
    idx_d = nc.dram_tensor("idx", [128, NT], I32, kind="ExternalInput").ap()
    wq_d = nc.dram_tensor("wq", [128, 3, H], BF16, kind="ExternalInput").ap()
    wk_d = nc.dram_tensor("wk", [128, 3, H], BF16, kind="ExternalInput").ap()
    watt_d = nc.dram_tensor("watt", [128, DT, G], BF16, kind="ExternalInput").ap()
    wout_d = nc.dram_tensor("wout", [128, G * DT, N_OUT], BF16,
                            kind="ExternalInput").ap()
    bout_d = nc.dram_tensor("bout", [1, N_OUT], BF16, kind="ExternalInput").ap()
    glove_d = nc.dram_tensor("glove", [128, 3, NA_PAD], BF16,
                             kind="ExternalInput").ap()
    ident_d = nc.dram_tensor("ident", [128, 128], BF16, kind="ExternalInput").ap()
    onesf_d = nc.dram_tensor("ones_f", [128, 1], F32, kind="ExternalInput").ap()
    onesb_d = nc.dram_tensor("ones_b", [1, BL], BF16, kind="ExternalInput").ap()
    out_d = nc.dram_tensor("out", [128, NA_T, BL], F32, kind="ExternalOutput").ap()
    warm_d = nc.dram_tensor("warm", [1, 128], F32, kind="ExternalOutput").ap()

    with tile.TileContext(nc) as tc:
        import contextlib

        with contextlib.ExitStack() as ctx:
            consts = ctx.enter_context(tc.tile_pool(name="consts", bufs=1))
            xrow_p = ctx.enter_context(tc.tile_pool(name="xrow", bufs=3))
            hqw_p = ctx.enter_context(tc.tile_pool(name="hqw", bufs=2))
            et_p = ctx.enter_context(tc.tile_pool(name="et", bufs=2))
            zz_p = ctx.enter_context(tc.tile_pool(name="zz", bufs=2))
            v_p = ctx.enter_context(tc.tile_pool(name="v", bufs=2))
            mm_p = ctx.enter_context(tc.tile_pool(name="mm", bufs=4, space="PSUM"))
            lg_p = ctx.enter_context(tc.tile_pool(name="lg", bufs=2, space="PSUM"))
            up_p = ctx.enter_context(tc.tile_pool(name="up", bufs=2, space="PSUM"))

            # ---- indices first so gathers start immediately ----
            idx_sb = consts.tile([128, NT], I32, tag="idx")
            nc.sync.dma_start(idx_sb[:], idx_d)

            # ---- PE warm-up on the identity while DMAs stream in ----
            ident = consts.tile([128, 128], BF16, tag="ident")
            nc.sync.dma_start(ident[:], ident_d)
            wps = lg_p.tile([128, 512], F32, tag="lg")
            for _ in range(24):
                nc.tensor.matmul(wps[:, :128], lhsT=ident[:], rhs=ident[:],
                                 start=True, stop=True)

            warm_sb = consts.tile([1, 128], F32, tag="warm")
            nc.vector.tensor_copy(warm_sb[:], wps[:1, :128])
            nc.sync.dma_start(warm_d, warm_sb[:])

            def keep_warm(n):
                """Filler ident matmuls: soak PE wait time during gathers so
                the HAM clock gate never re-throttles."""
                for _ in range(n):
                    nc.tensor.matmul(wps[:, :128], lhsT=ident[:], rhs=ident[:],
                                     start=True, stop=True)
            wq_sb = consts.tile([128, 3, H], BF16, tag="wq")
            nc.sync.dma_start(wq_sb[:], wq_d)
            wk_sb = consts.tile([128, 3, H], BF16, tag="wk")
            nc.sync.dma_start(wk_sb[:], wk_d)
            watt_sb = consts.tile([128, DT, G], BF16, tag="watt")
            nc.sync.dma_start(watt_sb[:], watt_d)
            bout_sb = consts.tile([1, N_OUT], BF16, tag="bout")
            nc.sync.dma_start(bout_sb[:], bout_d)
            onesf_sb = consts.tile([128, 1], F32, tag="onesf")
            nc.sync.dma_start(onesf_sb[:], onesf_d)
            onesb_sb = consts.tile([1, BL], BF16, tag="onesb")
            nc.sync.dma_start(onesb_sb[:], onesb_d)

            # weight streams for phases F/G (DMA engines fill during compute)
            wout_sb = consts.tile([128, G * DT, N_OUT], BF16, tag="wout")
            for t0 in range(0, G * DT, 16):
                nc.sync.dma_start(wout_sb[:, t0 : t0 + 16, :],
                                  wout_d[:, t0 : t0 + 16, :])
            glove_sb = consts.tile([128, 3, NA_PAD], BF16, tag="glove")
            for a0 in range(0, NA_PAD, 2048):
                nc.sync.dma_start(glove_sb[:, :, a0 : a0 + 2048],
                                  glove_d[:, :, a0 : a0 + 2048])

            # scratch that phases F/G need zeroed
            out_sb = consts.tile([16, EAP], BF16, tag="out_sb")
            nc.gpsimd.memset(out_sb[:], 0)
            shift_sb = consts.tile([128, 1], F32, tag="shift")
            nc.gpsimd.memset(shift_sb[:], SIM_SHIFT)
            e_sb = consts.tile([128, NA_T, BL], F32, tag="e_sb")
            nc.gpsimd.memset(e_sb[:, NA_T - 1, :], 0)

            # ---- gather token rows + PE transpose into X.T ----
            # xT[p, t, c, j] = emb[tok(t,j), c*128+p]
            xT = consts.tile([128, NT, 3, 128], BF16, tag="xT")
            for t in range(NT):
                xrow = xrow_p.tile([128, EAP], BF16, tag="xrow")
                nc.gpsimd.indirect_dma_start(
                    out=xrow[:],
                    out_offset=None,
                    in_=emb_d,
                    in_offset=bass.IndirectOffsetOnAxis(
                        ap=idx_sb[:, t : t + 1], axis=0
                    ),
                )
                keep_warm(14 if t < 4 else (6 if t < 8 else 2))
                ps_t = mm_p.tile([128, 512], BF16, tag="mm")
                for c in range(3):
                    nc.tensor.transpose(
                        ps_t[:, c * 128 : (c + 1) * 128],
                        xrow[:, c * 128 : (c + 1) * 128],
                        ident[:],
                    )
                nc.scalar.copy(
                    xT[:, t, :, :],
                    ps_t[:, :384].rearrange("p (c j) -> p c j", c=3),
                )

            # ---- hqT [128, DT, 256] ----
            hqT = consts.tile([128, DT, TQ], BF16, tag="hqT")
            for mp in range(DT // 2):
                ps = mm_p.tile([128, 512], F32, tag="mm")
                for mi in range(2):
                    m = mp * 2 + mi
                    for c in range(3):
                        nc.tensor.matmul(
                            ps[:, mi * 256 : (mi + 1) * 256],
                            lhsT=wq_sb[:, c, m * 128 : (m + 1) * 128],
                            rhs=xT[:, 0:2, c, :],
                            start=(c == 0),
                            stop=(c == 2),
                        )
                nc.scalar.copy(hqT[:, mp * 2 : mp * 2 + 2, :],
                               ps[:].rearrange("p (m q) -> p m q", m=2))

            hk = consts.tile([128, TK_TILES, H], BF16, tag="hk")
            hkT = consts.tile([128, TK_TILES, DT, 128], BF16, tag="hkT")
            poT = consts.tile([128, DT, G, BL], BF16, tag="poT")

            def emit_hk(t):
                """hk tile t = X[t] @ Wk  (token-partition layout)."""
                ps0 = mm_p.tile([128, 512], F32, tag="mm")
                ps1 = mm_p.tile([128, 512], F32, tag="mm")
                ps = [ps0, ps1]
                for c in range(3):
                    for h in range(2):
                        nc.tensor.matmul(
                            ps[h][:],
                            lhsT=xT[:, TQ_TILES + t, c, :],
                            rhs=wk_sb[:, c, h * 512 : (h + 1) * 512],
                            start=(c == 0),
                            stop=(c == 2),
                        )
                nc.scalar.copy(hk[:, t, 0:512], ps[0][:])
                nc.vector.tensor_copy(hk[:, t, 512:1024], ps[1][:])

            def emit_hkT(t0):
                """PE transpose of hk tiles t0..t0+3 into hkT."""
                for t in range(t0, t0 + 4):
                    for half in range(2):
                        ps_t = mm_p.tile([128, 512], BF16, tag="mm")
                        for mi in range(4):
                            m = half * 4 + mi
                            nc.tensor.transpose(
                                ps_t[:, mi * 128 : (mi + 1) * 128],
                                hk[:, t, m * 128 : (m + 1) * 128],
                                ident[:],
                            )
                        nc.vector.tensor_copy(
                            hkT[:, t, half * 4 : half * 4 + 4, :],
                            ps_t[:].rearrange("p (m j) -> p m j", m=4),
                        )

            def emit_attention(b):
                # hqw = hqT(b) * Watt  (broadcast over g), bf16 on DVE
                hqw = hqw_p.tile([128, DT, G, NQ], BF16, tag="hqw")
                with nc.allow_low_precision(reason="bf16 round of bf16 mult"):
                    nc.vector.tensor_tensor(
                        out=hqw[:],
                        in0=hqT[:, :, None, b * NQ : (b + 1) * NQ].to_broadcast(
                            [128, DT, G, NQ]
                        ),
                        in1=watt_sb[:, :, :, None].to_broadcast([128, DT, G, NQ]),
                        op=OP.mult,
                    )

                # logits.T [k, (kt, g, q)] in PSUM
                ps_l = lg_p.tile([128, 512], F32, tag="lg")
                for kt in range(2):
                    for m in range(DT):
                        nc.tensor.matmul(
                            ps_l[:, kt * 256 : (kt + 1) * 256],
                            lhsT=hkT[:, 2 * b + kt, m, :],
                            rhs=hqw[:, m],
                            start=(m == 0),
                            stop=(m == DT - 1),
                        )

                # E = exp(logits) (bf16), per-(kt,g) row sums zz (f32)
                et = et_p.tile([128, 2, 256], BF16, tag="et")
                zz = zz_p.tile([128, 2, G], F32, tag="zz")
                for kt in range(2):
                    nc.scalar.activation(
                        out=et[:, kt, :],
                        in_=ps_l[:, kt * 256 : (kt + 1) * 256],
                        func=AF.Exp,
                    )
                    nc.vector.tensor_reduce(
                        out=zz[:, kt, :],
                        in_=et[:, kt].rearrange("p (g q) -> p g q", g=G),
                        axis=AX.X,
                        op=OP.add,
                    )

                # Z_g = sum over k-partitions (fp32 PE), then 1/Z broadcast
                ps_z = lg_p.tile([128, 512], F32, tag="lg")
                for kt in range(2):
                    nc.tensor.matmul(
                        ps_z[:1, :G],
                        lhsT=onesf_sb[:],
                        rhs=zz[:, kt, :],
                        start=(kt == 0),
                        stop=(kt == 1),
                    )
                zinv = zz_p.tile([1, G], F32, tag="zinv")
                nc.vector.reciprocal(zinv[:1, :], ps_z[:1, :G])
                zbro = zz_p.tile([128, G], F32, tag="zbro")
                nc.gpsimd.partition_broadcast(zbro[:], zinv[:1, :], channels=128)

                # u = E.T-contract; v = u * hqT; pooled.T scaled by 1/Z
                vr = v_p.tile([128, DT, G], F32, tag="vr")
                for grp in range(4):
                    ps_u = up_p.tile([128, 512], F32, tag="up")
                    for mi in range(2):
                        m = grp * 2 + mi
                        for kt in range(2):
                            nc.tensor.matmul(
                                ps_u[:, mi * 256 : (mi + 1) * 256],
                                lhsT=hk[:, 2 * b + kt, m * 128 : (m + 1) * 128],
                                rhs=et[:, kt, :],
                                start=(kt == 0),
                                stop=(kt == 1),
                            )
                    v = v_p.tile([128, 2, G, NQ], BF16, tag="v")
                    with nc.allow_low_precision(reason="bf16 round"):
                        nc.vector.tensor_tensor(
                            out=v[:],
                            in0=ps_u[:].rearrange("p (m g q) -> p m g q",
                                                  m=2, g=G),
                            in1=hqT[
                                :, grp * 2 : grp * 2 + 2, None,
                                b * NQ : (b + 1) * NQ,
                            ].to_broadcast([128, 2, G, NQ]),
                            op=OP.mult,
                        )
                    nc.vector.tensor_reduce(
                        out=vr[:, grp * 2 : grp * 2 + 2, :], in_=v[:],
                        axis=AX.X, op=OP.add)
                with nc.allow_low_precision(reason="bf16 round"):
                    nc.vector.tensor_tensor(
                        out=poT[:, :, :, b],
                        in0=vr[:],
                        in1=zbro[:, None, :].to_broadcast([128, DT, G]),
                        op=OP.mult,
                    )

            # ---- interleave hk production with per-sample attention ----
            for t in range(4):
                emit_hk(t)
            emit_hkT(0)
            emit_attention(0)
            emit_attention(1)
            for t in range(4, 8):
                emit_hk(t)
            emit_hkT(4)
            emit_attention(2)
            emit_attention(3)
            for t in range(8, 12):
                emit_hk(t)
            emit_hkT(8)
            emit_attention(4)
            emit_attention(5)
            for t in range(12, 16):
                emit_hk(t)
            emit_hkT(12)
            emit_attention(6)
            emit_attention(7)

            # ---- phase F: out [8, 300] = pooled_flat @ Wout + bout ----
            ps_o = mm_p.tile([128, 512], F32, tag="mm")
            for g in range(G):
                for m in range(DT):
                    t = g * DT + m
                    nc.tensor.matmul(
                        ps_o[:BL, :N_OUT],
                        lhsT=poT[:, m, g, :],
                        rhs=wout_sb[:, t, :],
                        start=(t == 0),
                        stop=False,
                    )
            nc.tensor.matmul(
                ps_o[:BL, :N_OUT],
                lhsT=onesb_sb[:],
                rhs=bout_sb[:],
                start=False,
                stop=True,
            )
            with nc.allow_low_precision(reason="bf16 round of out"):
                nc.vector.tensor_copy(out_sb[0:BL, 0:N_OUT], ps_o[:BL, :N_OUT])
            outT = consts.tile([128, 3, 16], BF16, tag="outT")
            ps_ot = mm_p.tile([128, 512], BF16, tag="mm")
            for c in range(3):
                nc.tensor.transpose(
                    ps_ot[:, c * 16 : (c + 1) * 16],
                    out_sb[:, c * 128 : (c + 1) * 128],
                    ident[:16, :16],
                )
            nc.vector.tensor_copy(
                outT[:], ps_ot[:, :48].rearrange("p (c j) -> p c j", c=3))

            # ---- phase G: sim.T via glove-stationary matmuls ----
            ps_s = mm_p.tile([128, 512], F32, tag="mm")
            simT = ps_s[:, : NA_T * BL].rearrange("p (a b) -> p a b", a=NA_T)
            for a in range(NA_T):
                for c in range(3):
                    nc.tensor.matmul(
                        simT[:, a, :],
                        lhsT=glove_sb[:, c, a * 128 : (a + 1) * 128],
                        rhs=outT[:, c, 0:BL],
                        start=(c == 0),
                        stop=(c == 2),
                    )

            # log-softmax over (a-tile, partition) per sample column
            ntail = N_ANS - 128 * (NA_T - 1)
            nc.scalar.activation(out=e_sb[:, 0 : NA_T - 1, :],
                                 in_=simT[:, 0 : NA_T - 1, :],
                                 func=AF.Exp, bias=shift_sb[:])
            nc.scalar.activation(out=e_sb[0:ntail, NA_T - 1, :],
                                 in_=simT[0:ntail, NA_T - 1, :],
                                 func=AF.Exp, bias=shift_sb[0:ntail])
            ps_zt = lg_p.tile([128, 512], F32, tag="lg")
            nc.tensor.matmul(
                ps_zt[:1, : NA_T * BL],
                lhsT=onesf_sb[:],
                rhs=e_sb[:].rearrange("p a b -> p (a b)"),
                start=True,
                stop=True,
            )
            zrow = consts.tile([1, BL], F32, tag="zrow")
            nc.vector.tensor_reduce(
                out=zrow[:],
                in_=ps_zt[:1, : NA_T * BL].rearrange("p (a b) -> p b a", a=NA_T),
                axis=AX.X,
                op=OP.add,
            )
            lnz = consts.tile([1, BL], F32, tag="lnz")
            nc.scalar.activation(out=lnz[:], in_=zrow[:], func=AF.Ln)
            lnzb = consts.tile([128, BL], F32, tag="lnzb")
            nc.gpsimd.partition_broadcast(lnzb[:], lnz[:1, :], channels=128)

            # logp = (sim + SHIFT) - ln Z_shifted
            res_sb = consts.tile([128, NA_T, BL], F32, tag="res")
            nc.vector.scalar_tensor_tensor(
                out=res_sb[:],
                in0=simT[:],
                scalar=SIM_SHIFT,
                in1=lnzb[:, None, :].to_broadcast([128, NA_T, BL]),
                op0=OP.add,
                op1=OP.subtract,
            )
            nc.sync.dma_start(out_d, res_sb[:])

    nc.compile()
    return nc


_NC = None


def _get_nc():
    global _NC
    if _NC is None:
        _NC = build_kernel()
    return _NC


def make_in_maps(inputs):
    bf = ml_dtypes.bfloat16
    he_q = np.asarray(inputs["he_ques"]).astype(np.int32)   # [64, 32]
    he_k = np.asarray(inputs["he_kg"]).astype(np.int32)     # [64, 256]
    emb0 = np.asarray(inputs["emb"], dtype=np.float32)
    emb = np.zeros((VOCAB, EAP), dtype=bf)
    emb[:, :E] = emb0.astype(bf)
    emb[:, E] = 1.0

    def proj(wname, bname):
        w = np.zeros((EAP, H), dtype=np.float32)
        w[:E] = np.asarray(inputs[wname], np.float32)
        w[E] = np.asarray(inputs[bname], np.float32)
        return np.ascontiguousarray(
            w.reshape(3, 128, H).transpose(1, 0, 2)).astype(bf)

    wq = proj("Wq", "bq")
    wk = proj("Wk", "bk")
    watt = np.ascontiguousarray(
        np.asarray(inputs["Watt"], np.float32).reshape(DT, 128, G)
        .transpose(1, 0, 2)).astype(bf)                     # [128, DT, G]
    wout = np.ascontiguousarray(
        np.asarray(inputs["Wout"], np.float32).reshape(G * DT, 128, N_OUT)
        .transpose(1, 0, 2)).astype(bf)                     # [128, 64, 300]
    bout = np.asarray(inputs["bout"], np.float32).reshape(1, N_OUT).astype(bf)
    glove = np.zeros((EAP, NA_PAD), dtype=np.float32)
    glove[:N_OUT, :N_ANS] = np.asarray(inputs["glove_cands"], np.float32).T
    glove = np.ascontiguousarray(
        glove.reshape(3, 128, NA_PAD).transpose(1, 0, 2)).astype(bf)
    ident = np.eye(128, dtype=np.float32).astype(bf)

    in_maps = []
    for i in range(N_CORES):
        iq = he_q[i * BL : (i + 1) * BL].reshape(-1)        # [256]
        ik = he_k[i * BL : (i + 1) * BL].reshape(-1)        # [2048]
        idx = np.concatenate([iq, ik]).reshape(NT, 128).T   # [128, 18]
        in_maps.append({
            "emb": emb,
            "idx": np.ascontiguousarray(idx),
            "wq": wq,
            "wk": wk,
            "watt": watt,
            "wout": wout,
            "bout": bout,
            "glove": glove,
            "ident": ident,
            "ones_f": np.ones((128, 1), dtype=np.float32),
            "ones_b": np.ones((1, BL), dtype=bf),
        })
    return in_maps


def postprocess(res):
    """[128, 32, 8] simT-logp -> [8, 4000] log-probs."""
    r = np.asarray(res, dtype=np.float32)
    return r.transpose(2, 1, 0).reshape(BL, NA_PAD)[:, :N_ANS]


def kernel(**inputs) -> np.ndarray:
    nc = _get_nc()
    in_maps = make_in_maps(inputs)
    res = run_bass_kernel_spmd(nc, in_maps, list(range(N_CORES)))
    return np.concatenate(
        [postprocess(res.results[i]["out"]) for i in range(N_CORES)], axis=0
    )


# revision 24
# speedup vs baseline: 1.0121x; 1.0121x over previous
"""Trainium2 Bass kernel for the BAN (bilinear attention network) problem.

Math (per batch b, eval mode):
    hq = emb[he_ques] @ Wq + bq                  [NQ, H]
    hk = emb[he_kg]   @ Wk + bk                  [NK, H]
    logits[g,q,k] = sum_d hq[q,d] Watt[d,g] hk[k,d]   (+ batt[g], cancels in
                                                       the joint softmax)
    att = softmax over flattened (q,k) per (b,g)
    pooled[g,d] = sum_{q,k} hq[q,d] att[g,q,k] hk[k,d]
    out = pooled.flat @ Wout + bout;  sim = out @ glove.T;  log_softmax(sim)

Distribution: pure data parallel over batch, 8 samples per core on 8 cores.
All weights replicated. No collectives.

v2 design notes (vs the fp32r baseline):
  - All matmul operands are bf16.  fp32/fp32r stationary operands force a
    slow serial weight load into the PE for every matmul (measured ~200ns
    each, 157us total); bf16 enables FWL and pull-ahead so matmuls run at
    ~N-cycle streaming cost.  Accumulation stays fp32 in PSUM.  Measured
    headroom: tolerance is 2e-2, fp32r baseline error was 8.8e-5.
  - All transposes (X rows -> X.T, hk -> hk.T, out -> out.T) are PE
    transposes in bf16 (1 cycle/row + FWL weight loads), batched 4-to-a-PSUM
    tile so PSUM->SBUF copies are [128, 512] DVE 2x copies.  (The DMA XBAR
    transpose measured ~0.4us per 16x128 tile on HW -- 25x the cost-model
    estimate -- and serialized the whole kernel; the 2-column indirect
    gather also returned wrong data on HW.  Both were reverted.)
  - The final sim matrix is computed *transposed* ([a-tile partitions, batch]
    via glove-stationary matmuls) so log-softmax runs on 128 partitions
    instead of 8 (the [8, 4000] layout was partition-starved).
  - E (=300, +1 bias row) is zero-padded to 384 so every contraction chunk is
    a full 128 rows; N_ANS is padded 4000->4096 (pad exp terms masked to 0).

Layouts (per core, BL=8 samples, partition dim first):
  xT   [128, 18, 3, 128]  X.T     (e-in-chunk, tok-tile, e-chunk, tok)
  hqT  [128, 8, 256]      hq.T    (d-in-tile, m, q-tok)
  hk   [128, 16, 1024]    hk      (k-tok, k-tile, d)
  hkT  [128, 16, 8, 128]  hk.T    (d-in-tile, k-tile, m, k-tok)
  hqw  [128, 8, 8, 32]    hq.T*Watt broadcast over g     (per sample)
  et   [128, 2, 256]      exp(logits.T)  (k-tok, kt, (g,q))
  poT  [128, 8, 8, 8]     pooled.T (d-in-tile, m, g, b)
  simT [128, 32, 8] PSUM  sim.T   (ans-in-tile, ans-tile, b)
"""

import sys

if "/opt/trn_rl_repo" not in sys.path:
    sys.path.insert(0, "/opt/trn_rl_repo")

import numpy as np
import ml_dtypes

import concourse.bass as bass
import concourse.mybir as mybir
import concourse.tile as tile
from concourse import bacc
from concourse.bass_utils import run_bass_kernel_spmd

F32 = mybir.dt.float32
BF16 = mybir.dt.bfloat16
I32 = mybir.dt.int32
AX = mybir.AxisListType
OP = mybir.AluOpType
AF = mybir.ActivationFunctionType

N_CORES = 8
VOCAB = 20000
E = 300          # word embedding size
EAP = 384        # padded (bias row at 300, zeros 301..383); 3 chunks of 128
H = 1024         # hidden
G = 8            # heads
N_OUT = 300
N_ANS = 4000
NA_PAD = 4096    # padded answers; 32 tiles of 128
NA_T = 32
B, NQ, NK = 64, 32, 256
BL = B // N_CORES            # 8 samples per core
TQ = BL * NQ                 # 256 q tokens per core (2 tiles)
TK = BL * NK                 # 2048 k tokens per core (16 tiles)
TQ_TILES = TQ // 128         # 2
TK_TILES = TK // 128         # 16
NT = TQ_TILES + TK_TILES     # 18 gathered token tiles
DT = H // 128                # 8 d-tiles
SIM_SHIFT = -40.0            # exp(sim + SIM_SHIFT): overflow guard


def build_kernel():
    nc = bacc.Bacc("TRN2", target_bir_lowering=False, debug=False,
                   num_devices=N_CORES)

    # ---- DRAM I/O ----
    emb_d = nc.dram_tensor("emb", [VOCAB, EAP], BF16, kind="ExternalInput").ap()
    idx_d = nc.dram_tensor("idx", [128, NT], I32, kind="ExternalInput").ap()
    wq_d = nc.dram_tensor("wq", [128, 3, H], BF16, kind="ExternalInput").ap()
    wk_d = nc.dram_tensor("wk", [128, 3, H], BF16, kind="ExternalInput").ap()
    watt_d = nc.dram_tensor("watt", [128, DT, G], BF16, kind="ExternalInput").ap()
    wout_d = nc.dram_tensor("wout", [128, G * DT, N_OUT], BF16,
                            kind="ExternalInput").ap()
    bout_d = nc.dram_tensor("bout", [1, N_OUT], BF16, kind="ExternalInput").ap()
    glove_d = nc.dram_tensor("glove", [128, 3, NA_PAD], BF16,
                             kind="ExternalInput").ap()
    ident_d = nc.dram_tensor("ident", [128, 128], BF16, kind="ExternalInput").ap()
    onesf_d = nc.dram_tensor("ones_f", [128, 1], F32, kind="ExternalInput").ap()
    onesb_d = nc.dram_tensor("ones_b", [1, BL], BF16, kind="ExternalInput").ap()
    out_d = nc.dram_tensor("out", [128, NA_T, BL], F32, kind="ExternalOutput").ap()
    warm_d = nc.dram_tensor("warm", [1, 128], F32, kind="ExternalOutput").ap()

    with tile.TileContext(nc) as tc:
        import contextlib

        with contextlib.ExitStack() as ctx:
            consts = ctx.enter_context(tc.tile_pool(name="consts", bufs=1))
            xrow_p = ctx.enter_context(tc.tile_pool(name="xrow", bufs=8))
            et_p = ctx.enter_context(tc.tile_pool(name="et", bufs=2))
            zz_p = ctx.enter_context(tc.tile_pool(name="zz", bufs=2))
            v_p = ctx.enter_context(tc.tile_pool(name="v", bufs=2))
            mm_p = ctx.enter_context(tc.tile_pool(name="mm", bufs=4, space="PSUM"))
            lg_p = ctx.enter_context(tc.tile_pool(name="lg", bufs=2, space="PSUM"))
            up_p = ctx.enter_context(tc.tile_pool(name="up", bufs=2, space="PSUM"))

            # ---- indices first so gathers start immediately ----
            idx_sb = consts.tile([128, NT], I32, tag="idx")
            nc.sync.dma_start(idx_sb[:], idx_d)

            # ---- PE warm-up on the identity while DMAs stream in ----
            ident = consts.tile([128, 128], BF16, tag="ident")
            nc.sync.dma_start(ident[:], ident_d)
            wps = lg_p.tile([128, 512], F32, tag="lg")
            for _ in range(24):
                nc.tensor.matmul(wps[:, :128], lhsT=ident[:], rhs=ident[:],
                                 start=True, stop=True)

            warm_sb = consts.tile([1, 128], F32, tag="warm")
            nc.vector.tensor_copy(warm_sb[:], wps[:1, :128])
            nc.sync.dma_start(warm_d, warm_sb[:])

            def keep_warm(n):
                """Filler ident matmuls: soak PE wait time during gathers so
                the HAM clock gate never re-throttles."""
                for _ in range(n):
                    nc.tensor.matmul(wps[:, :128], lhsT=ident[:], rhs=ident[:],
                                     start=True, stop=True)
            wq_sb = consts.tile([128, 3, H], BF16, tag="wq")
            nc.sync.dma_start(wq_sb[:], wq_d)
            wk_sb = consts.tile([128, 3, H], BF16, tag="wk")
            nc.sync.dma_start(wk_sb[:], wk_d)
            watt_sb = consts.tile([128, DT, G], BF16, tag="watt")
            nc.sync.dma_start(watt_sb[:], watt_d)
            bout_sb = consts.tile([1, N_OUT], BF16, tag="bout")
            nc.sync.dma_start(bout_sb[:], bout_d)
            onesf_sb = consts.tile([128, 1], F32, tag="onesf")
            nc.sync.dma_start(onesf_sb[:], onesf_d)
            onesb_sb = consts.tile([1, BL], BF16, tag="onesb")
            nc.sync.dma_start(onesb_sb[:], onesb_d)

            # weight streams for phases F/G (DMA engines fill during compute)
            wout_sb = consts.tile([128, G * DT, N_OUT], BF16, tag="wout")
            for t0 in range(0, G * DT, 16):
                nc.sync.dma_start(wout_sb[:, t0 : t0 + 16, :],
                                  wout_d[:, t0 : t0 + 16, :])
            glove_sb = consts.tile([128, 3, NA_PAD], BF16, tag="glove")
            for a0 in range(0, NA_PAD, 2048):
                nc.sync.dma_start(glove_sb[:, :, a0 : a0 + 2048],
                                  glove_d[:, :, a0 : a0 + 2048])

            # scratch that phases F/G need zeroed
            out_sb = consts.tile([16, EAP], BF16, tag="out_sb")
            nc.gpsimd.memset(out_sb[:], 0)
            shift_sb = consts.tile([128, 1], F32, tag="shift")
            nc.gpsimd.memset(shift_sb[:], SIM_SHIFT)
            e_sb = consts.tile([128, NA_T, BL], F32, tag="e_sb")
            nc.gpsimd.memset(e_sb[:, NA_T - 1, :], 0)

            # ---- gather token rows + PE transpose into X.T ----
            # xT[p, t, c, j] = emb[tok(t,j), c*128+p]
            xT = consts.tile([128, NT, 3, 128], BF16, tag="xT")
            for t in range(NT):
                xrow = xrow_p.tile([128, EAP], BF16, tag="xrow")
                nc.gpsimd.indirect_dma_start(
                    out=xrow[:],
                    out_offset=None,
                    in_=emb_d,
                    in_offset=bass.IndirectOffsetOnAxis(
                        ap=idx_sb[:, t : t + 1], axis=0
                    ),
                )
                keep_warm(10 if t < 6 else 3)
                ps_t = mm_p.tile([128, 512], BF16, tag="mm")
                for c in range(3):
                    nc.tensor.transpose(
                        ps_t[:, c * 128 : (c + 1) * 128],
                        xrow[:, c * 128 : (c + 1) * 128],
                        ident[:],
                    )
                nc.scalar.copy(
                    xT[:, t, :, :],
                    ps_t[:, :384].rearrange("p (c j) -> p c j", c=3),
                )

            # ---- hqT [128, DT, 256] ----
            hqT = consts.tile([128, DT, TQ], BF16, tag="hqT")
            for mp in range(DT // 2):
                ps = mm_p.tile([128, 512], F32, tag="mm")
                for mi in range(2):
                    m = mp * 2 + mi
                    for c in range(3):
                        nc.tensor.matmul(
                            ps[:, mi * 256 : (mi + 1) * 256],
                            lhsT=wq_sb[:, c, m * 128 : (m + 1) * 128],
                            rhs=xT[:, 0:2, c, :],
                            start=(c == 0),
                            stop=(c == 2),
                        )
                nc.scalar.copy(hqT[:, mp * 2 : mp * 2 + 2, :],
                               ps[:].rearrange("p (m q) -> p m q", m=2))

            # hqw for ALL samples, computed while DVE is otherwise idle:
            # hqw[d, m, g, q_tok] = hqT[d, m, q_tok] * watt[d, m, g]
            hqw_all = consts.tile([128, DT, G, TQ], BF16, tag="hqw_all")
            for half in range(2):
                q0 = half * (TQ // 2)
                with nc.allow_low_precision(reason="bf16 round of bf16 mult"):
                    nc.vector.tensor_tensor(
                        out=hqw_all[:, :, :, q0 : q0 + TQ // 2],
                        in0=hqT[:, :, None, q0 : q0 + TQ // 2].to_broadcast(
                            [128, DT, G, TQ // 2]
                        ),
                        in1=watt_sb[:, :, :, None].to_broadcast(
                            [128, DT, G, TQ // 2]
                        ),
                        op=OP.mult,
                    )

            hk = consts.tile([128, TK_TILES, H], BF16, tag="hk")
            hkT = consts.tile([128, TK_TILES, DT, 128], BF16, tag="hkT")
            poT = consts.tile([128, DT, G, BL], BF16, tag="poT")

            def emit_hk(t):
                """hk tile t = X[t] @ Wk  (token-partition layout)."""
                ps0 = mm_p.tile([128, 512], F32, tag="mm")
                ps1 = mm_p.tile([128, 512], F32, tag="mm")
                ps = [ps0, ps1]
                for c in range(3):
                    for h in range(2):
                        nc.tensor.matmul(
                            ps[h][:],
                            lhsT=xT[:, TQ_TILES + t, c, :],
                            rhs=wk_sb[:, c, h * 512 : (h + 1) * 512],
                            start=(c == 0),
                            stop=(c == 2),
                        )
                nc.scalar.copy(hk[:, t, 0:512], ps[0][:])
                nc.vector.tensor_copy(hk[:, t, 512:1024], ps[1][:])

            def emit_hkT(t0):
                """PE transpose of hk tiles t0..t0+3 into hkT."""
                for t in range(t0, t0 + 4):
                    for half in range(2):
                        ps_t = mm_p.tile([128, 512], BF16, tag="mm")
                        for mi in range(4):
                            m = half * 4 + mi
                            nc.tensor.transpose(
                                ps_t[:, mi * 128 : (mi + 1) * 128],
                                hk[:, t, m * 128 : (m + 1) * 128],
                                ident[:],
                            )
                        dst = hkT[:, t, half * 4 : half * 4 + 4, :]
                        src = ps_t[:].rearrange("p (m j) -> p m j", m=4)
                        if half == 0:
                            nc.vector.tensor_copy(dst, src)
                        else:
                            nc.scalar.copy(dst, src)

            def emit_attention(b):
                # logits.T [k, (kt, g, q)] in PSUM
                ps_l = lg_p.tile([128, 512], F32, tag="lg")
                for kt in range(2):
                    for m in range(DT):
                        nc.tensor.matmul(
                            ps_l[:, kt * 256 : (kt + 1) * 256],
                            lhsT=hkT[:, 2 * b + kt, m, :],
                            rhs=hqw_all[:, m, :, b * NQ : (b + 1) * NQ],
                            start=(m == 0),
                            stop=(m == DT - 1),
                        )

                # E = exp(logits) (bf16), per-(kt,g) row sums zz (f32)
                et = et_p.tile([128, 2, 256], BF16, tag="et")
                zz = zz_p.tile([128, 2, G], F32, tag="zz")
                for kt in range(2):
                    nc.scalar.activation(
                        out=et[:, kt, :],
                        in_=ps_l[:, kt * 256 : (kt + 1) * 256],
                        func=AF.Exp,
                    )
                    nc.vector.tensor_reduce(
                        out=zz[:, kt, :],
                        in_=et[:, kt].rearrange("p (g q) -> p g q", g=G),
                        axis=AX.X,
                        op=OP.add,
                    )

                # Z_g = sum over k-partitions (fp32 PE), then 1/Z broadcast
                ps_z = lg_p.tile([128, 512], F32, tag="lg")
                for kt in range(2):
                    nc.tensor.matmul(
                        ps_z[:1, :G],
                        lhsT=onesf_sb[:],
                        rhs=zz[:, kt, :],
                        start=(kt == 0),
                        stop=(kt == 1),
                    )
                zinv = zz_p.tile([1, G], F32, tag="zinv")
                nc.vector.reciprocal(zinv[:1, :], ps_z[:1, :G])
                zbro = zz_p.tile([128, G], F32, tag="zbro")
                nc.gpsimd.partition_broadcast(zbro[:], zinv[:1, :], channels=128)

                # u = E.T-contract; v = u * hqT; pooled.T scaled by 1/Z
                vr = v_p.tile([128, DT, G], F32, tag="vr")
                for grp in range(4):
                    ps_u = up_p.tile([128, 512], F32, tag="up")
                    for mi in range(2):
                        m = grp * 2 + mi
                        for kt in range(2):
                            nc.tensor.matmul(
                                ps_u[:, mi * 256 : (mi + 1) * 256],
                                lhsT=hk[:, 2 * b + kt, m * 128 : (m + 1) * 128],
                                rhs=et[:, kt, :],
                                start=(kt == 0),
                                stop=(kt == 1),
                            )
                    v = v_p.tile([128, 2, G, NQ], BF16, tag="v")
                    with nc.allow_low_precision(reason="bf16 round"):
                        nc.vector.tensor_tensor(
                            out=v[:],
                            in0=ps_u[:].rearrange("p (m g q) -> p m g q",
                                                  m=2, g=G),
                            in1=hqT[
                                :, grp * 2 : grp * 2 + 2, None,
                                b * NQ : (b + 1) * NQ,
                            ].to_broadcast([128, 2, G, NQ]),
                            op=OP.mult,
                        )
                    nc.vector.tensor_reduce(
                        out=vr[:, grp * 2 : grp * 2 + 2, :], in_=v[:],
                        axis=AX.X, op=OP.add)
                with nc.allow_low_precision(reason="bf16 round"):
                    nc.vector.tensor_tensor(
                        out=poT[:, :, :, b],
                        in0=vr[:],
                        in1=zbro[:, None, :].to_broadcast([128, DT, G]),
                        op=OP.mult,
                    )

            # ---- interleave hk production with per-sample attention ----
            for t in range(4):
                emit_hk(t)
            emit_hkT(0)
            emit_attention(0)
            emit_attention(1)
            for t in range(4, 8):
                emit_hk(t)
            emit_hkT(4)
            emit_attention(2)
            emit_attention(3)
            for t in range(8, 12):
                emit_hk(t)
            emit_hkT(8)
            emit_attention(4)
            emit_attention(5)
            for t in range(12, 16):
                emit_hk(t)
            emit_hkT(12)
            emit_attention(6)
            emit_attention(7)

            # ---- phase F: out [8, 300] = pooled_flat @ Wout + bout ----
            ps_o = mm_p.tile([128, 512], F32, tag="mm")
            for g in range(G):
                for m in range(DT):
                    t = g * DT + m
                    nc.tensor.matmul(
                        ps_o[:BL, :N_OUT],
                        lhsT=poT[:, m, g, :],
                        rhs=wout_sb[:, t, :],
                        start=(t == 0),
                        stop=False,
                    )
            nc.tensor.matmul(
                ps_o[:BL, :N_OUT],
                lhsT=onesb_sb[:],
                rhs=bout_sb[:],
                start=False,
                stop=True,
            )
            with nc.allow_low_precision(reason="bf16 round of out"):
                nc.vector.tensor_copy(out_sb[0:BL, 0:N_OUT], ps_o[:BL, :N_OUT])
            outT = consts.tile([128, 3, 16], BF16, tag="outT")
            ps_ot = mm_p.tile([128, 512], BF16, tag="mm")
            for c in range(3):
                nc.tensor.transpose(
                    ps_ot[:, c * 16 : (c + 1) * 16],
                    out_sb[:, c * 128 : (c + 1) * 128],
                    ident[:16, :16],
                )
            nc.vector.tensor_copy(
                outT[:], ps_ot[:, :48].rearrange("p (c j) -> p c j", c=3))

            # ---- phase G: sim.T via glove-stationary matmuls ----
            ps_s = mm_p.tile([128, 512], F32, tag="mm")
            simT = ps_s[:, : NA_T * BL].rearrange("p (a b) -> p a b", a=NA_T)
            for a in range(NA_T):
                for c in range(3):
                    nc.tensor.matmul(
                        simT[:, a, :],
                        lhsT=glove_sb[:, c, a * 128 : (a + 1) * 128],
                        rhs=outT[:, c, 0:BL],
                        start=(c == 0),
                        stop=(c == 2),
                    )

            # log-softmax over (a-tile, partition) per sample column
            ntail = N_ANS - 128 * (NA_T - 1)
            nc.scalar.activation(out=e_sb[:, 0 : NA_T - 1, :],
                                 in_=simT[:, 0 : NA_T - 1, :],
                                 func=AF.Exp, bias=shift_sb[:])
            nc.scalar.activation(out=e_sb[0:ntail, NA_T - 1, :],
                                 in_=simT[0:ntail, NA_T - 1, :],
                                 func=AF.Exp, bias=shift_sb[0:ntail])
            ps_zt = lg_p.tile([128, 512], F32, tag="lg")
            nc.tensor.matmul(
                ps_zt[:1, : NA_T * BL],
                lhsT=onesf_sb[:],
                rhs=e_sb[:].rearrange("p a b -> p (a b)"),
                start=True,
                stop=True,
            )
            zrow = consts.tile([1, BL], F32, tag="zrow")
            nc.vector.tensor_reduce(
                out=zrow[:],
                in_=ps_zt[:1, : NA_T * BL].rearrange("p (a b) -> p b a", a=NA_T),
                axis=AX.X,
                op=OP.add,
            )
            lnz = consts.tile([1, BL], F32, tag="lnz")
            nc.scalar.activation(out=lnz[:], in_=zrow[:], func=AF.Ln)
            lnzb = consts.tile([128, BL], F32, tag="lnzb")
            nc.gpsimd.partition_broadcast(lnzb[:], lnz[:1, :], channels=128)

            # logp = (sim + SHIFT) - ln Z_shifted
            res_sb = consts.tile([128, NA_T, BL], F32, tag="res")
            nc.vector.scalar_tensor_tensor(
                out=res_sb[:],
                in0=simT[:],
                scalar=SIM_SHIFT,
                in1=lnzb[:, None, :].to_broadcast([128, NA_T, BL]),
                op0=OP.add,
                op1=OP.subtract,
            )
            nc.sync.dma_start(out_d, res_sb[:])

    nc.compile()
    return nc


_NC = None


def _get_nc():
    global _NC
    if _NC is None:
        _NC = build_kernel()
    return _NC


def make_in_maps(inputs):
    bf = ml_dtypes.bfloat16
    he_q = np.asarray(inputs["he_ques"]).astype(np.int32)   # [64, 32]
    he_k = np.asarray(inputs["he_kg"]).astype(np.int32)     # [64, 256]
    emb0 = np.asarray(inputs["emb"], dtype=np.float32)
    emb = np.zeros((VOCAB, EAP), dtype=bf)
    emb[:, :E] = emb0.astype(bf)
    emb[:, E] = 1.0

    def proj(wname, bname):
        w = np.zeros((EAP, H), dtype=np.float32)
        w[:E] = np.asarray(inputs[wname], np.float32)
        w[E] = np.asarray(inputs[bname], np.float32)
        return np.ascontiguousarray(
            w.reshape(3, 128, H).transpose(1, 0, 2)).astype(bf)

    wq = proj("Wq", "bq")
    wk = proj("Wk", "bk")
    watt = np.ascontiguousarray(
        np.asarray(inputs["Watt"], np.float32).reshape(DT, 128, G)
        .transpose(1, 0, 2)).astype(bf)                     # [128, DT, G]
    wout = np.ascontiguousarray(
        np.asarray(inputs["Wout"], np.float32).reshape(G * DT, 128, N_OUT)
        .transpose(1, 0, 2)).astype(bf)                     # [128, 64, 300]
    bout = np.asarray(inputs["bout"], np.float32).reshape(1, N_OUT).astype(bf)
    glove = np.zeros((EAP, NA_PAD), dtype=np.float32)
    glove[:N_OUT, :N_ANS] = np.asarray(inputs["glove_cands"], np.float32).T
    glove = np.ascontiguousarray(
        glove.reshape(3, 128, NA_PAD).transpose(1, 0, 2)).astype(bf)
    ident = np.eye(128, dtype=np.float32).astype(bf)

    in_maps = []
    for i in range(N_CORES):
        iq = he_q[i * BL : (i + 1) * BL].reshape(-1)        # [256]
        ik = he_k[i * BL : (i + 1) * BL].reshape(-1)        # [2048]
        idx = np.concatenate([iq, ik]).reshape(NT, 128).T   # [128, 18]
        in_maps.append({
            "emb": emb,
            "idx": np.ascontiguousarray(idx),
            "wq": wq,
            "wk": wk,
            "watt": watt,
            "wout": wout,
            "bout": bout,
            "glove": glove,
            "ident": ident,
            "ones_f": np.ones((128, 1), dtype=np.float32),
            "ones_b": np.ones((1, BL), dtype=bf),
        })
    return in_maps


def postprocess(res):
    """[128, 32, 8] simT-logp -> [8, 4000] log-probs."""
    r = np.asarray(res, dtype=np.float32)
    return r.transpose(2, 1, 0).reshape(BL, NA_PAD)[:, :N_ANS]


def kernel(**inputs) -> np.ndarray:
    nc = _get_nc()
    in_maps = make_in_maps(inputs)
    res = run_bass_kernel_spmd(nc, in_maps, list(range(N_CORES)))
    return np.concatenate(
        [postprocess(res.results[i]["out"]) for i in range(N_CORES)], axis=0
    )


# revision 26
# speedup vs baseline: 1.1938x; 1.1795x over previous
"""Trainium2 Bass kernel for the BAN (bilinear attention network) problem.

Math (per batch b, eval mode):
    hq = emb[he_ques] @ Wq + bq                  [NQ, H]
    hk = emb[he_kg]   @ Wk + bk                  [NK, H]
    logits[g,q,k] = sum_d hq[q,d] Watt[d,g] hk[k,d]   (+ batt[g], cancels in
                                                       the joint softmax)
    att = softmax over flattened (q,k) per (b,g)
    pooled[g,d] = sum_{q,k} hq[q,d] att[g,q,k] hk[k,d]
    out = pooled.flat @ Wout + bout;  sim = out @ glove.T;  log_softmax(sim)

Distribution: pure data parallel over batch, 8 samples per core on 8 cores.
All weights replicated. No collectives.

v2 design notes (vs the fp32r baseline):
  - All matmul operands are bf16.  fp32/fp32r stationary operands force a
    slow serial weight load into the PE for every matmul (measured ~200ns
    each, 157us total); bf16 enables FWL and pull-ahead so matmuls run at
    ~N-cycle streaming cost.  Accumulation stays fp32 in PSUM.  Measured
    headroom: tolerance is 2e-2, fp32r baseline error was 8.8e-5.
  - All transposes (X rows -> X.T, hk -> hk.T, out -> out.T) are PE
    transposes in bf16 (1 cycle/row + FWL weight loads), batched 4-to-a-PSUM
    tile so PSUM->SBUF copies are [128, 512] DVE 2x copies.  (The DMA XBAR
    transpose measured ~0.4us per 16x128 tile on HW -- 25x the cost-model
    estimate -- and serialized the whole kernel; the 2-column indirect
    gather also returned wrong data on HW.  Both were reverted.)
  - The final sim matrix is computed *transposed* ([a-tile partitions, batch]
    via glove-stationary matmuls) so log-softmax runs on 128 partitions
    instead of 8 (the [8, 4000] layout was partition-starved).
  - E (=300, +1 bias row) is zero-padded to 384 so every contraction chunk is
    a full 128 rows; N_ANS is padded 4000->4096 (pad exp terms masked to 0).

Layouts (per core, BL=8 samples, partition dim first):
  xT   [128, 18, 3, 128]  X.T     (e-in-chunk, tok-tile, e-chunk, tok)
  hqT  [128, 8, 256]      hq.T    (d-in-tile, m, q-tok)
  hk   [128, 16, 1024]    hk      (k-tok, k-tile, d)
  hkT  [128, 16, 8, 128]  hk.T    (d-in-tile, k-tile, m, k-tok)
  hqw  [128, 8, 8, 32]    hq.T*Watt broadcast over g     (per sample)
  et   [128, 2, 256]      exp(logits.T)  (k-tok, kt, (g,q))
  poT  [128, 8, 8, 8]     pooled.T (d-in-tile, m, g, b)
  simT [128, 32, 8] PSUM  sim.T   (ans-in-tile, ans-tile, b)
"""

import sys

if "/opt/trn_rl_repo" not in sys.path:
    sys.path.insert(0, "/opt/trn_rl_repo")

import numpy as np
import ml_dtypes

import concourse.bass as bass
import concourse.mybir as mybir
import concourse.tile as tile
from concourse import bacc
from concourse.bass_utils import run_bass_kernel_spmd

F32 = mybir.dt.float32
BF16 = mybir.dt.bfloat16
I32 = mybir.dt.int32
AX = mybir.AxisListType
OP = mybir.AluOpType
AF = mybir.ActivationFunctionType

N_CORES = 8
VOCAB = 20000
E = 300          # word embedding size
EAP = 384        # padded (bias row at 300, zeros 301..383); 3 chunks of 128
H = 1024         # hidden
G = 8            # heads
N_OUT = 300
N_ANS = 4000
NA_PAD = 4096    # padded answers; 32 tiles of 128
NA_T = 32
B, NQ, NK = 64, 32, 256
BL = B // N_CORES            # 8 samples per core
TQ = BL * NQ                 # 256 q tokens per core (2 tiles)
TK = BL * NK                 # 2048 k tokens per core (16 tiles)
TQ_TILES = TQ // 128         # 2
TK_TILES = TK // 128         # 16
NT = TQ_TILES + TK_TILES     # 18 gathered token tiles
DT = H // 128                # 8 d-tiles
SIM_SHIFT = -40.0            # exp(sim + SIM_SHIFT): overflow guard


def build_kernel():
    nc = bacc.Bacc("TRN2", target_bir_lowering=False, debug=False,
                   num_devices=N_CORES)

    # ---- DRAM I/O ----
    emb_d = nc.dram_tensor("emb", [VOCAB, EAP], BF16, kind="ExternalInput").ap()
    idx_d = nc.dram_tensor("idx", [128, NT], I32, kind="ExternalInput").ap()
    wq_d = nc.dram_tensor("wq", [128, 3, H], BF16, kind="ExternalInput").ap()
    wk_d = nc.dram_tensor("wk", [128, 3, H], BF16, kind="ExternalInput").ap()
    watt_d = nc.dram_tensor("watt", [128, DT, G], BF16, kind="ExternalInput").ap()
    wout_d = nc.dram_tensor("wout", [128, G * DT, N_OUT], BF16,
                            kind="ExternalInput").ap()
    bout_d = nc.dram_tensor("bout", [1, N_OUT], BF16, kind="ExternalInput").ap()
    glove_d = nc.dram_tensor("glove", [128, 3, NA_PAD], BF16,
                             kind="ExternalInput").ap()
    ident_d = nc.dram_tensor("ident", [128, 128], BF16, kind="ExternalInput").ap()
    onesf_d = nc.dram_tensor("ones_f", [128, 1], F32, kind="ExternalInput").ap()
    onesb_d = nc.dram_tensor("ones_b", [1, BL], BF16, kind="ExternalInput").ap()
    out_d = nc.dram_tensor("out", [128, NA_T, BL], F32, kind="ExternalOutput").ap()
    warm_d = nc.dram_tensor("warm", [1, 128], F32, kind="ExternalOutput").ap()

    with tile.TileContext(nc) as tc:
        import contextlib

        with contextlib.ExitStack() as ctx:
            consts = ctx.enter_context(tc.tile_pool(name="consts", bufs=1))
            xrow_p = ctx.enter_context(tc.tile_pool(name="xrow", bufs=8))
            et_p = ctx.enter_context(tc.tile_pool(name="et", bufs=2))
            zz_p = ctx.enter_context(tc.tile_pool(name="zz", bufs=2))
            v_p = ctx.enter_context(tc.tile_pool(name="v", bufs=2))
            mm_p = ctx.enter_context(tc.tile_pool(name="mm", bufs=4, space="PSUM"))
            lg_p = ctx.enter_context(tc.tile_pool(name="lg", bufs=2, space="PSUM"))
            up_p = ctx.enter_context(tc.tile_pool(name="up", bufs=2, space="PSUM"))

            # ---- indices first so gathers start immediately ----
            idx_sb = consts.tile([128, NT], I32, tag="idx")
            nc.sync.dma_start(idx_sb[:], idx_d)

            # ---- PE warm-up on the identity while DMAs stream in ----
            ident = consts.tile([128, 128], BF16, tag="ident")
            nc.sync.dma_start(ident[:], ident_d)
            wps = lg_p.tile([128, 512], F32, tag="lg")
            for _ in range(24):
                nc.tensor.matmul(wps[:, :128], lhsT=ident[:], rhs=ident[:],
                                 start=True, stop=True)

            warm_sb = consts.tile([1, 128], F32, tag="warm")
            nc.vector.tensor_copy(warm_sb[:], wps[:1, :128])
            nc.sync.dma_start(warm_d, warm_sb[:])

            def keep_warm(n):
                """Filler ident matmuls: soak PE wait time during gathers so
                the HAM clock gate never re-throttles."""
                for _ in range(n):
                    nc.tensor.matmul(wps[:, :128], lhsT=ident[:], rhs=ident[:],
                                     start=True, stop=True)
            wq_sb = consts.tile([128, 3, H], BF16, tag="wq")
            nc.sync.dma_start(wq_sb[:], wq_d)
            wk_sb = consts.tile([128, 3, H], BF16, tag="wk")
            nc.sync.dma_start(wk_sb[:], wk_d)
            watt_sb = consts.tile([128, DT, G], BF16, tag="watt")
            nc.sync.dma_start(watt_sb[:], watt_d)
            bout_sb = consts.tile([1, N_OUT], BF16, tag="bout")
            nc.sync.dma_start(bout_sb[:], bout_d)
            onesf_sb = consts.tile([128, 1], F32, tag="onesf")
            nc.sync.dma_start(onesf_sb[:], onesf_d)
            onesb_sb = consts.tile([1, BL], BF16, tag="onesb")
            nc.sync.dma_start(onesb_sb[:], onesb_d)

            # weight streams for phases F/G (DMA engines fill during compute)
            wout_sb = consts.tile([128, G * DT, N_OUT], BF16, tag="wout")
            for t0 in range(0, G * DT, 16):
                nc.sync.dma_start(wout_sb[:, t0 : t0 + 16, :],
                                  wout_d[:, t0 : t0 + 16, :])
            glove_sb = consts.tile([128, 3, NA_PAD], BF16, tag="glove")
            for a0 in range(0, NA_PAD, 2048):
                nc.sync.dma_start(glove_sb[:, :, a0 : a0 + 2048],
                                  glove_d[:, :, a0 : a0 + 2048])

            # scratch that phases F/G need zeroed
            out_sb = consts.tile([16, EAP], BF16, tag="out_sb")
            nc.gpsimd.memset(out_sb[:], 0)
            shift_sb = consts.tile([128, 1], F32, tag="shift")
            nc.gpsimd.memset(shift_sb[:], SIM_SHIFT)
            e_sb = consts.tile([128, NA_T, BL], F32, tag="e_sb")
            nc.gpsimd.memset(e_sb[:, NA_T - 1, :], 0)

            # ---- gather + PE transpose into X.T ----
            # xT[p, t, c, j] = emb[tok(t,j), c*128+p]
            xT = consts.tile([128, NT, 3, 128], BF16, tag="xT")

            def emit_gather(t):
                xrow = xrow_p.tile([128, EAP], BF16, tag="xrow")
                nc.gpsimd.indirect_dma_start(
                    out=xrow[:],
                    out_offset=None,
                    in_=emb_d,
                    in_offset=bass.IndirectOffsetOnAxis(
                        ap=idx_sb[:, t : t + 1], axis=0
                    ),
                )
                return xrow

            def emit_xT(t, xrow):
                ps_t = mm_p.tile([128, 512], BF16, tag="mm")
                for c in range(3):
                    nc.tensor.transpose(
                        ps_t[:, c * 128 : (c + 1) * 128],
                        xrow[:, c * 128 : (c + 1) * 128],
                        ident[:],
                    )
                nc.scalar.copy(
                    xT[:, t, :, :],
                    ps_t[:, :384].rearrange("p (c j) -> p c j", c=3),
                )

            # q tiles first: everything q-side is needed by every sample
            xrow_q = [emit_gather(t) for t in range(TQ_TILES)]
            keep_warm(20)
            for t in range(TQ_TILES):
                emit_xT(t, xrow_q[t])

            # ---- hqT [128, DT, 256] ----
            hqT = consts.tile([128, DT, TQ], BF16, tag="hqT")
            for mp in range(DT // 2):
                ps = mm_p.tile([128, 512], F32, tag="mm")
                for mi in range(2):
                    m = mp * 2 + mi
                    for c in range(3):
                        nc.tensor.matmul(
                            ps[:, mi * 256 : (mi + 1) * 256],
                            lhsT=wq_sb[:, c, m * 128 : (m + 1) * 128],
                            rhs=xT[:, 0:2, c, :],
                            start=(c == 0),
                            stop=(c == 2),
                        )
                nc.scalar.copy(hqT[:, mp * 2 : mp * 2 + 2, :],
                               ps[:].rearrange("p (m q) -> p m q", m=2))

            # hqw for ALL samples on DVE, early (overlaps the k gathers):
            # hqw[d, m, g, q_tok] = hqT[d, m, q_tok] * watt[d, m, g]
            hqw_all = consts.tile([128, DT, G, TQ], BF16, tag="hqw_all")
            for quarter in range(4):
                q0 = quarter * (TQ // 4)
                with nc.allow_low_precision(reason="bf16 round of bf16 mult"):
                    nc.vector.tensor_tensor(
                        out=hqw_all[:, :, :, q0 : q0 + TQ // 4],
                        in0=hqT[:, :, None, q0 : q0 + TQ // 4].to_broadcast(
                            [128, DT, G, TQ // 4]
                        ),
                        in1=watt_sb[:, :, :, None].to_broadcast(
                            [128, DT, G, TQ // 4]
                        ),
                        op=OP.mult,
                    )

            hk = consts.tile([128, TK_TILES, H], BF16, tag="hk")
            hkT = consts.tile([128, TK_TILES, DT, 128], BF16, tag="hkT")
            poT = consts.tile([128, DT, G, BL], BF16, tag="poT")

            def emit_hk(t):
                """hk tile t = X[t] @ Wk  (token-partition layout)."""
                ps0 = mm_p.tile([128, 512], F32, tag="mm")
                ps1 = mm_p.tile([128, 512], F32, tag="mm")
                ps = [ps0, ps1]
                for c in range(3):
                    for h in range(2):
                        nc.tensor.matmul(
                            ps[h][:],
                            lhsT=xT[:, TQ_TILES + t, c, :],
                            rhs=wk_sb[:, c, h * 512 : (h + 1) * 512],
                            start=(c == 0),
                            stop=(c == 2),
                        )
                nc.scalar.copy(hk[:, t, 0:512], ps[0][:])
                nc.vector.tensor_copy(hk[:, t, 512:1024], ps[1][:])

            def emit_hkT(t0):
                """PE transpose of hk tiles t0..t0+3 into hkT."""
                for t in range(t0, t0 + 4):
                    for half in range(2):
                        ps_t = mm_p.tile([128, 512], BF16, tag="mm")
                        for mi in range(4):
                            m = half * 4 + mi
                            nc.tensor.transpose(
                                ps_t[:, mi * 128 : (mi + 1) * 128],
                                hk[:, t, m * 128 : (m + 1) * 128],
                                ident[:],
                            )
                        dst = hkT[:, t, half * 4 : half * 4 + 4, :]
                        src = ps_t[:].rearrange("p (m j) -> p m j", m=4)
                        if half == 0:
                            nc.vector.tensor_copy(dst, src)
                        else:
                            nc.scalar.copy(dst, src)

            def emit_attention(b):
                # logits.T [k, (kt, g, q)] in PSUM
                ps_l = lg_p.tile([128, 512], F32, tag="lg")
                for kt in range(2):
                    for m in range(DT):
                        nc.tensor.matmul(
                            ps_l[:, kt * 256 : (kt + 1) * 256],
                            lhsT=hkT[:, 2 * b + kt, m, :],
                            rhs=hqw_all[:, m, :, b * NQ : (b + 1) * NQ],
                            start=(m == 0),
                            stop=(m == DT - 1),
                        )

                # E = exp(logits) (bf16), per-(kt,g) row sums zz (f32)
                et = et_p.tile([128, 2, 256], BF16, tag="et")
                zz = zz_p.tile([128, 2, G], F32, tag="zz")
                for kt in range(2):
                    nc.scalar.activation(
                        out=et[:, kt, :],
                        in_=ps_l[:, kt * 256 : (kt + 1) * 256],
                        func=AF.Exp,
                    )
                    nc.vector.tensor_reduce(
                        out=zz[:, kt, :],
                        in_=et[:, kt].rearrange("p (g q) -> p g q", g=G),
                        axis=AX.X,
                        op=OP.add,
                    )

                # Z_g = sum over k-partitions (fp32 PE), then 1/Z broadcast
                ps_z = lg_p.tile([128, 512], F32, tag="lg")
                for kt in range(2):
                    nc.tensor.matmul(
                        ps_z[:1, :G],
                        lhsT=onesf_sb[:],
                        rhs=zz[:, kt, :],
                        start=(kt == 0),
                        stop=(kt == 1),
                    )
                zinv = zz_p.tile([1, G], F32, tag="zinv")
                nc.vector.reciprocal(zinv[:1, :], ps_z[:1, :G])
                zbro = zz_p.tile([128, G], F32, tag="zbro")
                nc.gpsimd.partition_broadcast(zbro[:], zinv[:1, :], channels=128)

                # u = E.T-contract; v = u * hqT; pooled.T scaled by 1/Z
                vr = v_p.tile([128, DT, G], F32, tag="vr")
                for grp in range(4):
                    ps_u = up_p.tile([128, 512], F32, tag="up")
                    for mi in range(2):
                        m = grp * 2 + mi
                        for kt in range(2):
                            nc.tensor.matmul(
                                ps_u[:, mi * 256 : (mi + 1) * 256],
                                lhsT=hk[:, 2 * b + kt, m * 128 : (m + 1) * 128],
                                rhs=et[:, kt, :],
                                start=(kt == 0),
                                stop=(kt == 1),
                            )
                    v = v_p.tile([128, 2, G, NQ], BF16, tag="v")
                    with nc.allow_low_precision(reason="bf16 round"):
                        nc.vector.tensor_tensor(
                            out=v[:],
                            in0=ps_u[:].rearrange("p (m g q) -> p m g q",
                                                  m=2, g=G),
                            in1=hqT[
                                :, grp * 2 : grp * 2 + 2, None,
                                b * NQ : (b + 1) * NQ,
                            ].to_broadcast([128, 2, G, NQ]),
                            op=OP.mult,
                        )
                    nc.vector.tensor_reduce(
                        out=vr[:, grp * 2 : grp * 2 + 2, :], in_=v[:],
                        axis=AX.X, op=OP.add)
                with nc.allow_low_precision(reason="bf16 round"):
                    nc.vector.tensor_tensor(
                        out=poT[:, :, :, b],
                        in0=vr[:],
                        in1=zbro[:, None, :].to_broadcast([128, DT, G]),
                        op=OP.mult,
                    )

            # ---- interleave gather/transpose/hk with per-sample attention ----
            for t in range(TK_TILES):
                xrow = emit_gather(TQ_TILES + t)
                if t < 6:
                    keep_warm(4)
                emit_xT(TQ_TILES + t, xrow)
                emit_hk(t)
                if t % 4 == 3:
                    emit_hkT(t - 3)
                    emit_attention((t - 3) // 2)
                    emit_attention((t - 3) // 2 + 1)

            # ---- phase F: out [8, 300] = pooled_flat @ Wout + bout ----
            ps_o = mm_p.tile([128, 512], F32, tag="mm")
            for g in range(G):
                for m in range(DT):
                    t = g * DT + m
                    nc.tensor.matmul(
                        ps_o[:BL, :N_OUT],
                        lhsT=poT[:, m, g, :],
                        rhs=wout_sb[:, t, :],
                        start=(t == 0),
                        stop=False,
                    )
            nc.tensor.matmul(
                ps_o[:BL, :N_OUT],
                lhsT=onesb_sb[:],
                rhs=bout_sb[:],
                start=False,
                stop=True,
            )
            with nc.allow_low_precision(reason="bf16 round of out"):
                nc.vector.tensor_copy(out_sb[0:BL, 0:N_OUT], ps_o[:BL, :N_OUT])
            outT = consts.tile([128, 3, 16], BF16, tag="outT")
            ps_ot = mm_p.tile([128, 512], BF16, tag="mm")
            for c in range(3):
                nc.tensor.transpose(
                    ps_ot[:, c * 16 : (c + 1) * 16],
                    out_sb[:, c * 128 : (c + 1) * 128],
                    ident[:16, :16],
                )
            nc.vector.tensor_copy(
                outT[:], ps_ot[:, :48].rearrange("p (c j) -> p c j", c=3))

            # ---- phase G: sim.T via glove-stationary matmuls ----
            ps_s = mm_p.tile([128, 512], F32, tag="mm")
            simT = ps_s[:, : NA_T * BL].rearrange("p (a b) -> p a b", a=NA_T)
            for a in range(NA_T):
                for c in range(3):
                    nc.tensor.matmul(
                        simT[:, a, :],
                        lhsT=glove_sb[:, c, a * 128 : (a + 1) * 128],
                        rhs=outT[:, c, 0:BL],
                        start=(c == 0),
                        stop=(c == 2),
                    )

            # log-softmax over (a-tile, partition) per sample column
            ntail = N_ANS - 128 * (NA_T - 1)
            nc.scalar.activation(out=e_sb[:, 0 : NA_T - 1, :],
                                 in_=simT[:, 0 : NA_T - 1, :],
                                 func=AF.Exp, bias=shift_sb[:])
            nc.scalar.activation(out=e_sb[0:ntail, NA_T - 1, :],
                                 in_=simT[0:ntail, NA_T - 1, :],
                                 func=AF.Exp, bias=shift_sb[0:ntail])
            ps_zt = lg_p.tile([128, 512], F32, tag="lg")
            nc.tensor.matmul(
                ps_zt[:1, : NA_T * BL],
                lhsT=onesf_sb[:],
                rhs=e_sb[:].rearrange("p a b -> p (a b)"),
                start=True,
                stop=True,
            )
            zrow = consts.tile([1, BL], F32, tag="zrow")
            nc.vector.tensor_reduce(
                out=zrow[:],
                in_=ps_zt[:1, : NA_T * BL].rearrange("p (a b) -> p b a", a=NA_T),
                axis=AX.X,
                op=OP.add,
            )
            lnz = consts.tile([1, BL], F32, tag="lnz")
            nc.scalar.activation(out=lnz[:], in_=zrow[:], func=AF.Ln)
            lnzb = consts.tile([128, BL], F32, tag="lnzb")
            nc.gpsimd.partition_broadcast(lnzb[:], lnz[:1, :], channels=128)

            # logp = (sim + SHIFT) - ln Z_shifted
            res_sb = consts.tile([128, NA_T, BL], F32, tag="res")
            nc.vector.scalar_tensor_tensor(
                out=res_sb[:],
                in0=simT[:],
                scalar=SIM_SHIFT,
                in1=lnzb[:, None, :].to_broadcast([128, NA_T, BL]),
                op0=OP.add,
                op1=OP.subtract,
            )
            nc.sync.dma_start(out_d, res_sb[:])

    nc.compile()
    return nc


_NC = None


def _get_nc():
    global _NC
    if _NC is None:
        _NC = build_kernel()
    return _NC


def make_in_maps(inputs):
    bf = ml_dtypes.bfloat16
    he_q = np.asarray(inputs["he_ques"]).astype(np.int32)   # [64, 32]
    he_k = np.asarray(inputs["he_kg"]).astype(np.int32)     # [64, 256]
    emb0 = np.asarray(inputs["emb"], dtype=np.float32)
    emb = np.zeros((VOCAB, EAP), dtype=bf)
    emb[:, :E] = emb0.astype(bf)
    emb[:, E] = 1.0

    def proj(wname, bname):
        w = np.zeros((EAP, H), dtype=np.float32)
        w[:E] = np.asarray(inputs[wname], np.float32)
        w[E] = np.asarray(inputs[bname], np.float32)
        return np.ascontiguousarray(
            w.reshape(3, 128, H).transpose(1, 0, 2)).astype(bf)

    wq = proj("Wq", "bq")
    wk = proj("Wk", "bk")
    watt = np.ascontiguousarray(
        np.asarray(inputs["Watt"], np.float32).reshape(DT, 128, G)
        .transpose(1, 0, 2)).astype(bf)                     # [128, DT, G]
    wout = np.ascontiguousarray(
        np.asarray(inputs["Wout"], np.float32).reshape(G * DT, 128, N_OUT)
        .transpose(1, 0, 2)).astype(bf)                     # [128, 64, 300]
    bout = np.asarray(inputs["bout"], np.float32).reshape(1, N_OUT).astype(bf)
    glove = np.zeros((EAP, NA_PAD), dtype=np.float32)
    glove[:N_OUT, :N_ANS] = np.asarray(inputs["glove_cands"], np.float32).T
    glove = np.ascontiguousarray(
        glove.reshape(3, 128, NA_PAD).transpose(1, 0, 2)).astype(bf)
    ident = np.eye(128, dtype=np.float32).astype(bf)

    in_maps = []
    for i in range(N_CORES):
        iq = he_q[i * BL : (i + 1) * BL].reshape(-1)        # [256]
        ik = he_k[i * BL : (i + 1) * BL].reshape(-1)        # [2048]
        idx = np.concatenate([iq, ik]).reshape(NT, 128).T   # [128, 18]
        in_maps.append({
            "emb": emb,
            "idx": np.ascontiguousarray(idx),
            "wq": wq,
            "wk": wk,
            "watt": watt,
            "wout": wout,
            "bout": bout,
            "glove": glove,
            "ident": ident,
            "ones_f": np.ones((128, 1), dtype=np.float32),
            "ones_b": np.ones((1, BL), dtype=bf),
        })
    return in_maps


def postprocess(res):
    """[128, 32, 8] simT-logp -> [8, 4000] log-probs."""
    r = np.asarray(res, dtype=np.float32)
    return r.transpose(2, 1, 0).reshape(BL, NA_PAD)[:, :N_ANS]


def kernel(**inputs) -> np.ndarray:
    nc = _get_nc()
    in_maps = make_in_maps(inputs)
    res = run_bass_kernel_spmd(nc, in_maps, list(range(N_CORES)))
    return np.concatenate(
        [postprocess(res.results[i]["out"]) for i in range(N_CORES)], axis=0
    )


# revision 36
# speedup vs baseline: 1.2565x; 1.0525x over previous
"""Trainium2 Bass kernel for the BAN (bilinear attention network) problem.

Math (per batch b, eval mode):
    hq = emb[he_ques] @ Wq + bq                  [NQ, H]
    hk = emb[he_kg]   @ Wk + bk                  [NK, H]
    logits[g,q,k] = sum_d hq[q,d] Watt[d,g] hk[k,d]   (+ batt[g], cancels in
                                                       the joint softmax)
    att = softmax over flattened (q,k) per (b,g)
    pooled[g,d] = sum_{q,k} hq[q,d] att[g,q,k] hk[k,d]
    out = pooled.flat @ Wout + bout;  sim = out @ glove.T;  log_softmax(sim)

Distribution: pure data parallel over batch, 8 samples per core on 8 cores.
All weights replicated. No collectives.

v2 design notes (vs the fp32r baseline):
  - All matmul operands are bf16.  fp32/fp32r stationary operands force a
    slow serial weight load into the PE for every matmul (measured ~200ns
    each, 157us total); bf16 enables FWL and pull-ahead so matmuls run at
    ~N-cycle streaming cost.  Accumulation stays fp32 in PSUM.  Measured
    headroom: tolerance is 2e-2, fp32r baseline error was 8.8e-5.
  - All transposes (X rows -> X.T, hk -> hk.T, out -> out.T) are PE
    transposes in bf16 (1 cycle/row + FWL weight loads), batched 4-to-a-PSUM
    tile so PSUM->SBUF copies are [128, 512] DVE 2x copies.  (The DMA XBAR
    transpose measured ~0.4us per 16x128 tile on HW -- 25x the cost-model
    estimate -- and serialized the whole kernel; the 2-column indirect
    gather also returned wrong data on HW.  Both were reverted.)
  - The final sim matrix is computed *transposed* ([a-tile partitions, batch]
    via glove-stationary matmuls) so log-softmax runs on 128 partitions
    instead of 8 (the [8, 4000] layout was partition-starved).
  - E (=300, +1 bias row) is zero-padded to 384 so every contraction chunk is
    a full 128 rows; N_ANS is padded 4000->4096 (pad exp terms masked to 0).

Layouts (per core, BL=8 samples, partition dim first):
  xT   [128, 18, 3, 128]  X.T     (e-in-chunk, tok-tile, e-chunk, tok)
  hqT  [128, 8, 256]      hq.T    (d-in-tile, m, q-tok)
  hk   [128, 16, 1024]    hk      (k-tok, k-tile, d)
  hkT  [128, 16, 8, 128]  hk.T    (d-in-tile, k-tile, m, k-tok)
  hqw  [128, 8, 8, 32]    hq.T*Watt broadcast over g     (per sample)
  et   [128, 2, 256]      exp(logits.T)  (k-tok, kt, (g,q))
  poT  [128, 8, 8, 8]     pooled.T (d-in-tile, m, g, b)
  simT [128, 32, 8] PSUM  sim.T   (ans-in-tile, ans-tile, b)
"""

import sys

if "/opt/trn_rl_repo" not in sys.path:
    sys.path.insert(0, "/opt/trn_rl_repo")

import numpy as np
import ml_dtypes

import concourse.bass as bass
import concourse.mybir as mybir
import concourse.tile as tile
from concourse import bacc
from concourse.bass_utils import run_bass_kernel_spmd

F32 = mybir.dt.float32
BF16 = mybir.dt.bfloat16
I32 = mybir.dt.int32
AX = mybir.AxisListType
OP = mybir.AluOpType
AF = mybir.ActivationFunctionType

N_CORES = 8
VOCAB = 20000
E = 300          # word embedding size
EAP = 384        # padded (bias row at 300, zeros 301..383); 3 chunks of 128
H = 1024         # hidden
G = 8            # heads
N_OUT = 300
N_ANS = 4000
NA_PAD = 4096    # padded answers; 32 tiles of 128
NA_T = 32
B, NQ, NK = 64, 32, 256
BL = B // N_CORES            # 8 samples per core
TQ = BL * NQ                 # 256 q tokens per core (2 tiles)
TK = BL * NK                 # 2048 k tokens per core (16 tiles)
TQ_TILES = TQ // 128         # 2
TK_TILES = TK // 128         # 16
NT = TQ_TILES + TK_TILES     # 18 gathered token tiles
DT = H // 128                # 8 d-tiles
SIM_SHIFT = -40.0            # exp(sim + SIM_SHIFT): overflow guard


def build_kernel():
    nc = bacc.Bacc("TRN2", target_bir_lowering=False, debug=False,
                   num_devices=N_CORES)

    # ---- DRAM I/O ----
    emb_d = nc.dram_tensor("emb", [VOCAB, EAP], BF16, kind="ExternalInput").ap()
    idx_d = nc.dram_tensor("idx", [128, NT], I32, kind="ExternalInput").ap()
    wq_d = nc.dram_tensor("wq", [128, 3, H], BF16, kind="ExternalInput").ap()
    wk_d = nc.dram_tensor("wk", [128, 3, H], BF16, kind="ExternalInput").ap()
    watt_d = nc.dram_tensor("watt", [128, DT, G], BF16, kind="ExternalInput").ap()
    wout_d = nc.dram_tensor("wout", [128, G * DT, N_OUT], BF16,
                            kind="ExternalInput").ap()
    bout_d = nc.dram_tensor("bout", [1, N_OUT], BF16, kind="ExternalInput").ap()
    glove_d = nc.dram_tensor("glove", [128, 3, NA_PAD], BF16,
                             kind="ExternalInput").ap()
    ident_d = nc.dram_tensor("ident", [128, 128], BF16, kind="ExternalInput").ap()
    onesf_d = nc.dram_tensor("ones_f", [128, 1], F32, kind="ExternalInput").ap()
    onesb_d = nc.dram_tensor("ones_b", [1, BL], BF16, kind="ExternalInput").ap()
    out_d = nc.dram_tensor("out", [128, NA_T, BL], F32, kind="ExternalOutput").ap()
    warm_d = nc.dram_tensor("warm", [1, 128], F32, kind="ExternalOutput").ap()

    with tile.TileContext(nc) as tc:
        import contextlib

        with contextlib.ExitStack() as ctx:
            consts = ctx.enter_context(tc.tile_pool(name="consts", bufs=1))
            xrow_p = ctx.enter_context(tc.tile_pool(name="xrow", bufs=8))
            et_p = ctx.enter_context(tc.tile_pool(name="et", bufs=2))
            zz_p = ctx.enter_context(tc.tile_pool(name="zz", bufs=2))
            v_p = ctx.enter_context(tc.tile_pool(name="v", bufs=2))
            mm_p = ctx.enter_context(tc.tile_pool(name="mm", bufs=4, space="PSUM"))
            lg_p = ctx.enter_context(tc.tile_pool(name="lg", bufs=2, space="PSUM"))
            up_p = ctx.enter_context(tc.tile_pool(name="up", bufs=2, space="PSUM"))

            # ---- identity + indices first: warmup and gathers start early ----
            ident = consts.tile([128, 128], BF16, tag="ident")
            nc.sync.dma_start(ident[:], ident_d)
            idx_sb = consts.tile([128, NT], I32, tag="idx")
            nc.sync.dma_start(idx_sb[:], idx_d)
            wps = lg_p.tile([128, 512], F32, tag="lg")
            for _ in range(24):
                nc.tensor.matmul(wps[:, :128], lhsT=ident[:], rhs=ident[:],
                                 start=True, stop=True)

            warm_sb = consts.tile([1, 128], F32, tag="warm")
            nc.vector.tensor_copy(warm_sb[:], wps[:1, :128])
            nc.sync.dma_start(warm_d, warm_sb[:])

            def keep_warm(n):
                """Filler ident matmuls: soak PE wait time during gathers so
                the HAM clock gate never re-throttles."""
                for _ in range(n):
                    nc.tensor.matmul(wps[:, :128], lhsT=ident[:], rhs=ident[:],
                                     start=True, stop=True)
            wq_sb = consts.tile([128, 3, H], BF16, tag="wq")
            nc.sync.dma_start(wq_sb[:], wq_d)
            wk_sb = consts.tile([128, 3, H], BF16, tag="wk")
            nc.sync.dma_start(wk_sb[:], wk_d)
            watt_sb = consts.tile([128, DT, G], BF16, tag="watt")
            nc.sync.dma_start(watt_sb[:], watt_d)
            bout_sb = consts.tile([1, N_OUT], BF16, tag="bout")
            nc.sync.dma_start(bout_sb[:], bout_d)
            onesf_sb = consts.tile([128, 1], F32, tag="onesf")
            nc.sync.dma_start(onesf_sb[:], onesf_d)
            onesb_sb = consts.tile([1, BL], BF16, tag="onesb")
            nc.sync.dma_start(onesb_sb[:], onesb_d)

            # F/G weight tiles (DMAs emitted mid-attention so their transfers
            # don't compete with the startup gathers for DMA engines)
            wout_sb = consts.tile([128, G * DT, N_OUT], BF16, tag="wout")
            glove_sb = consts.tile([128, 3, NA_PAD], BF16, tag="glove")

            def emit_weight_streams():
                for t0 in range(0, G * DT, 16):
                    nc.sync.dma_start(wout_sb[:, t0 : t0 + 16, :],
                                      wout_d[:, t0 : t0 + 16, :])
                for a0 in range(0, NA_PAD, 2048):
                    nc.sync.dma_start(glove_sb[:, :, a0 : a0 + 2048],
                                      glove_d[:, :, a0 : a0 + 2048])

            # scratch that phases F/G need zeroed
            out_sb = consts.tile([16, EAP], BF16, tag="out_sb")
            nc.gpsimd.memset(out_sb[:], 0)
            shift_sb = consts.tile([128, 1], F32, tag="shift")
            nc.gpsimd.memset(shift_sb[:], SIM_SHIFT)
            e_sb = consts.tile([128, NA_T, BL], F32, tag="e_sb")
            nc.gpsimd.memset(e_sb[:, NA_T - 1, :], 0)

            # ---- gather + PE transpose into X.T ----
            # xT[p, t, c, j] = emb[tok(t,j), c*128+p]
            xT = consts.tile([128, NT, 3, 128], BF16, tag="xT")

            def emit_gather(t):
                xrow = xrow_p.tile([128, EAP], BF16, tag="xrow")
                nc.gpsimd.indirect_dma_start(
                    out=xrow[:],
                    out_offset=None,
                    in_=emb_d,
                    in_offset=bass.IndirectOffsetOnAxis(
                        ap=idx_sb[:, t : t + 1], axis=0
                    ),
                )
                return xrow

            def emit_xT(t, xrow):
                ps_t = mm_p.tile([128, 512], BF16, tag="mm")
                for c in range(3):
                    nc.tensor.transpose(
                        ps_t[:, c * 128 : (c + 1) * 128],
                        xrow[:, c * 128 : (c + 1) * 128],
                        ident[:],
                    )
                nc.scalar.copy(
                    xT[:, t, :, :],
                    ps_t[:, :384].rearrange("p (c j) -> p c j", c=3),
                )

            # q tiles first: everything q-side is needed by every sample
            xrow_q = [emit_gather(t) for t in range(TQ_TILES)]
            keep_warm(20)
            for t in range(TQ_TILES):
                emit_xT(t, xrow_q[t])

            # ---- hqT [128, DT, 256] ----
            hqT = consts.tile([128, DT, TQ], BF16, tag="hqT")
            for mp in range(DT // 2):
                ps = mm_p.tile([128, 512], F32, tag="mm")
                for mi in range(2):
                    m = mp * 2 + mi
                    for c in range(3):
                        nc.tensor.matmul(
                            ps[:, mi * 256 : (mi + 1) * 256],
                            lhsT=wq_sb[:, c, m * 128 : (m + 1) * 128],
                            rhs=xT[:, 0:2, c, :],
                            start=(c == 0),
                            stop=(c == 2),
                        )
                nc.scalar.copy(hqT[:, mp * 2 : mp * 2 + 2, :],
                               ps[:].rearrange("p (m q) -> p m q", m=2))

            # hqw[d, m, g, q_tok] = hqT[d, m, q_tok] * watt[d, m, g]; one
            # quarter (2 samples) emitted per attention round, in DVE slack
            hqw_all = consts.tile([128, DT, G, TQ], BF16, tag="hqw_all")

            def emit_hqw(quarter):
                q0 = quarter * (TQ // 4)
                with nc.allow_low_precision(reason="bf16 round of bf16 mult"):
                    nc.vector.tensor_tensor(
                        out=hqw_all[:, :, :, q0 : q0 + TQ // 4],
                        in0=hqT[:, :, None, q0 : q0 + TQ // 4].to_broadcast(
                            [128, DT, G, TQ // 4]
                        ),
                        in1=watt_sb[:, :, :, None].to_broadcast(
                            [128, DT, G, TQ // 4]
                        ),
                        op=OP.mult,
                    )

            emit_hqw(0)

            hk = consts.tile([128, TK_TILES, H], BF16, tag="hk")
            hkT = consts.tile([128, TK_TILES, DT, 128], BF16, tag="hkT")
            poT = consts.tile([128, DT, G, BL], BF16, tag="poT")

            def emit_hk(t):
                """hk tile t = X[t] @ Wk  (token-partition layout)."""
                ps0 = mm_p.tile([128, 512], F32, tag="mm")
                ps1 = mm_p.tile([128, 512], F32, tag="mm")
                ps = [ps0, ps1]
                for c in range(3):
                    for h in range(2):
                        nc.tensor.matmul(
                            ps[h][:],
                            lhsT=xT[:, TQ_TILES + t, c, :],
                            rhs=wk_sb[:, c, h * 512 : (h + 1) * 512],
                            start=(c == 0),
                            stop=(c == 2),
                        )
                nc.scalar.copy(hk[:, t, 0:512], ps[0][:])
                nc.scalar.copy(hk[:, t, 512:1024], ps[1][:])

            def emit_hkT(t0):
                """PE transpose of hk tiles t0..t0+3 into hkT."""
                for t in range(t0, t0 + 4):
                    for half in range(2):
                        ps_t = mm_p.tile([128, 512], BF16, tag="mm")
                        for mi in range(4):
                            m = half * 4 + mi
                            nc.tensor.transpose(
                                ps_t[:, mi * 128 : (mi + 1) * 128],
                                hk[:, t, m * 128 : (m + 1) * 128],
                                ident[:],
                            )
                        nc.vector.tensor_copy(
                            hkT[:, t, half * 4 : half * 4 + 4, :],
                            ps_t[:].rearrange("p (m j) -> p m j", m=4),
                        )

            def emit_attention(b):
                # logits.T [k, (kt, g, q)] in PSUM
                ps_l = lg_p.tile([128, 512], F32, tag="lg")
                for kt in range(2):
                    for m in range(DT):
                        nc.tensor.matmul(
                            ps_l[:, kt * 256 : (kt + 1) * 256],
                            lhsT=hkT[:, 2 * b + kt, m, :],
                            rhs=hqw_all[:, m, :, b * NQ : (b + 1) * NQ],
                            start=(m == 0),
                            stop=(m == DT - 1),
                        )

                # E = exp(logits) (bf16), per-(kt,g) row sums zz (f32)
                et = et_p.tile([128, 2, 256], BF16, tag="et")
                zz = zz_p.tile([128, 2, G], F32, tag="zz")
                for kt in range(2):
                    nc.scalar.activation(
                        out=et[:, kt, :],
                        in_=ps_l[:, kt * 256 : (kt + 1) * 256],
                        func=AF.Exp,
                    )
                    nc.vector.tensor_reduce(
                        out=zz[:, kt, :],
                        in_=et[:, kt].rearrange("p (g q) -> p g q", g=G),
                        axis=AX.X,
                        op=OP.add,
                    )

                # Z_g = sum over k-partitions (fp32 PE), then 1/Z broadcast
                ps_z = lg_p.tile([128, 512], F32, tag="lg")
                for kt in range(2):
                    nc.tensor.matmul(
                        ps_z[:1, :G],
                        lhsT=onesf_sb[:],
                        rhs=zz[:, kt, :],
                        start=(kt == 0),
                        stop=(kt == 1),
                    )
                zinv = zz_p.tile([1, G], F32, tag="zinv")
                nc.vector.reciprocal(zinv[:1, :], ps_z[:1, :G])
                zbro = zz_p.tile([128, G], F32, tag="zbro")
                nc.gpsimd.partition_broadcast(zbro[:], zinv[:1, :], channels=128)

                # u = E.T-contract; v = u * hqT; pooled.T scaled by 1/Z
                vr = v_p.tile([128, DT, G], BF16, tag="vr")
                for grp in range(4):
                    ps_u = up_p.tile([128, 512], F32, tag="up")
                    for mi in range(2):
                        m = grp * 2 + mi
                        for kt in range(2):
                            nc.tensor.matmul(
                                ps_u[:, mi * 256 : (mi + 1) * 256],
                                lhsT=hk[:, 2 * b + kt, m * 128 : (m + 1) * 128],
                                rhs=et[:, kt, :],
                                start=(kt == 0),
                                stop=(kt == 1),
                            )
                    v = v_p.tile([128, 2, G, NQ], BF16, tag="v")
                    with nc.allow_low_precision(reason="bf16 round"):
                        nc.vector.tensor_tensor(
                            out=v[:],
                            in0=ps_u[:].rearrange("p (m g q) -> p m g q",
                                                  m=2, g=G),
                            in1=hqT[
                                :, grp * 2 : grp * 2 + 2, None,
                                b * NQ : (b + 1) * NQ,
                            ].to_broadcast([128, 2, G, NQ]),
                            op=OP.mult,
                        )
                    with nc.allow_low_precision(reason="bf16 vr; pooled is bf16"):
                        nc.vector.tensor_reduce(
                            out=vr[:, grp * 2 : grp * 2 + 2, :], in_=v[:],
                            axis=AX.X, op=OP.add)
                with nc.allow_low_precision(reason="bf16 round"):
                    nc.vector.tensor_tensor(
                        out=poT[:, :, :, b],
                        in0=vr[:],
                        in1=zbro[:, None, :].to_broadcast([128, DT, G]),
                        op=OP.mult,
                    )

            # ---- interleave gather/transpose/hk with per-sample attention ----
            for t in range(TK_TILES):
                xrow = emit_gather(TQ_TILES + t)
                if t < 6:
                    keep_warm(4)
                emit_xT(TQ_TILES + t, xrow)
                emit_hk(t)
                if t % 4 == 3:
                    emit_hkT(t - 3)
                    if t == 3:
                        emit_weight_streams()
                    if t < TK_TILES - 1:
                        emit_hqw((t - 3) // 4 + 1)
                    emit_attention((t - 3) // 2)
                    emit_attention((t - 3) // 2 + 1)

            # ---- phase F: out [8, 300] = pooled_flat @ Wout + bout ----
            # fillers bridge the poT(b7) wait so F runs at full PE clock
            keep_warm(24)
            ps_o = mm_p.tile([128, 512], F32, tag="mm")
            for g in range(G):
                for m in range(DT):
                    t = g * DT + m
                    nc.tensor.matmul(
                        ps_o[:BL, :N_OUT],
                        lhsT=poT[:, m, g, :],
                        rhs=wout_sb[:, t, :],
                        start=(t == 0),
                        stop=False,
                    )
            nc.tensor.matmul(
                ps_o[:BL, :N_OUT],
                lhsT=onesb_sb[:],
                rhs=bout_sb[:],
                start=False,
                stop=True,
            )
            with nc.allow_low_precision(reason="bf16 round of out"):
                nc.vector.tensor_copy(out_sb[0:BL, 0:N_OUT], ps_o[:BL, :N_OUT])
            outT = consts.tile([128, 3, 16], BF16, tag="outT")
            ps_ot = mm_p.tile([128, 512], BF16, tag="mm")
            for c in range(3):
                nc.tensor.transpose(
                    ps_ot[:, c * 16 : (c + 1) * 16],
                    out_sb[:, c * 128 : (c + 1) * 128],
                    ident[:16, :16],
                )
            nc.vector.tensor_copy(
                outT[:], ps_ot[:, :48].rearrange("p (c j) -> p c j", c=3))

            # ---- phase G: sim.T via glove-stationary matmuls ----
            ps_s = mm_p.tile([128, 512], F32, tag="mm")
            simT = ps_s[:, : NA_T * BL].rearrange("p (a b) -> p a b", a=NA_T)
            for a in range(NA_T):
                for c in range(3):
                    nc.tensor.matmul(
                        simT[:, a, :],
                        lhsT=glove_sb[:, c, a * 128 : (a + 1) * 128],
                        rhs=outT[:, c, 0:BL],
                        start=(c == 0),
                        stop=(c == 2),
                    )

            # log-softmax over (a-tile, partition) per sample column
            ntail = N_ANS - 128 * (NA_T - 1)
            nc.scalar.activation(out=e_sb[:, 0 : NA_T - 1, :],
                                 in_=simT[:, 0 : NA_T - 1, :],
                                 func=AF.Exp, bias=shift_sb[:])
            nc.scalar.activation(out=e_sb[0:ntail, NA_T - 1, :],
                                 in_=simT[0:ntail, NA_T - 1, :],
                                 func=AF.Exp, bias=shift_sb[0:ntail])
            ps_zt = lg_p.tile([128, 512], F32, tag="lg")
            nc.tensor.matmul(
                ps_zt[:1, : NA_T * BL],
                lhsT=onesf_sb[:],
                rhs=e_sb[:].rearrange("p a b -> p (a b)"),
                start=True,
                stop=True,
            )
            zrow = consts.tile([1, BL], F32, tag="zrow")
            nc.vector.tensor_reduce(
                out=zrow[:],
                in_=ps_zt[:1, : NA_T * BL].rearrange("p (a b) -> p b a", a=NA_T),
                axis=AX.X,
                op=OP.add,
            )
            lnz = consts.tile([1, BL], F32, tag="lnz")
            nc.scalar.activation(out=lnz[:], in_=zrow[:], func=AF.Ln)
            lnzb = consts.tile([128, BL], F32, tag="lnzb")
            nc.gpsimd.partition_broadcast(lnzb[:], lnz[:1, :], channels=128)

            # logp = (sim + SHIFT) - ln Z_shifted
            res_sb = consts.tile([128, NA_T, BL], F32, tag="res")
            nc.vector.scalar_tensor_tensor(
                out=res_sb[:],
                in0=simT[:],
                scalar=SIM_SHIFT,
                in1=lnzb[:, None, :].to_broadcast([128, NA_T, BL]),
                op0=OP.add,
                op1=OP.subtract,
            )
            nc.sync.dma_start(out_d, res_sb[:])

    nc.compile()
    return nc


_NC = None


def _get_nc():
    global _NC
    if _NC is None:
        _NC = build_kernel()
    return _NC


def make_in_maps(inputs):
    bf = ml_dtypes.bfloat16
    he_q = np.asarray(inputs["he_ques"]).astype(np.int32)   # [64, 32]
    he_k = np.asarray(inputs["he_kg"]).astype(np.int32)     # [64, 256]
    emb0 = np.asarray(inputs["emb"], dtype=np.float32)
    emb = np.zeros((VOCAB, EAP), dtype=bf)
    emb[:, :E] = emb0.astype(bf)
    emb[:, E] = 1.0

    def proj(wname, bname):
        w = np.zeros((EAP, H), dtype=np.float32)
        w[:E] = np.asarray(inputs[wname], np.float32)
        w[E] = np.asarray(inputs[bname], np.float32)
        return np.ascontiguousarray(
            w.reshape(3, 128, H).transpose(1, 0, 2)).astype(bf)

    wq = proj("Wq", "bq")
    wk = proj("Wk", "bk")
    watt = np.ascontiguousarray(
        np.asarray(inputs["Watt"], np.float32).reshape(DT, 128, G)
        .transpose(1, 0, 2)).astype(bf)                     # [128, DT, G]
    wout = np.ascontiguousarray(
        np.asarray(inputs["Wout"], np.float32).reshape(G * DT, 128, N_OUT)
        .transpose(1, 0, 2)).astype(bf)                     # [128, 64, 300]
    bout = np.asarray(inputs["bout"], np.float32).reshape(1, N_OUT).astype(bf)
    glove = np.zeros((EAP, NA_PAD), dtype=np.float32)
    glove[:N_OUT, :N_ANS] = np.asarray(inputs["glove_cands"], np.float32).T
    glove = np.ascontiguousarray(
        glove.reshape(3, 128, NA_PAD).transpose(1, 0, 2)).astype(bf)
    ident = np.eye(128, dtype=np.float32).astype(bf)

    in_maps = []
    for i in range(N_CORES):
        iq = he_q[i * BL : (i + 1) * BL].reshape(-1)        # [256]
        ik = he_k[i * BL : (i + 1) * BL].reshape(-1)        # [2048]
        idx = np.concatenate([iq, ik]).reshape(NT, 128).T   # [128, 18]
        in_maps.append({
            "emb": emb,
            "idx": np.ascontiguousarray(idx),
            "wq": wq,
            "wk": wk,
            "watt": watt,
            "wout": wout,
            "bout": bout,
            "glove": glove,
            "ident": ident,
            "ones_f": np.ones((128, 1), dtype=np.float32),
            "ones_b": np.ones((1, BL), dtype=bf),
        })
    return in_maps


def postprocess(res):
    """[128, 32, 8] simT-logp -> [8, 4000] log-probs."""
    r = np.asarray(res, dtype=np.float32)
    return r.transpose(2, 1, 0).reshape(BL, NA_PAD)[:, :N_ANS]


def kernel(**inputs) -> np.ndarray:
    nc = _get_nc()
    in_maps = make_in_maps(inputs)
    res = run_bass_kernel_spmd(nc, in_maps, list(range(N_CORES)))
    return np.concatenate(
        [postprocess(res.results[i]["out"]) for i in range(N_CORES)], axis=0
    )
